# revision 18
# baseline (speedup 1.0000x reference)
"""MatchingNet forward on 8 Trainium2 NeuronCores (Bass/Tile).

Math (reference):
    s_emb = l2norm(support @ W + b)   [Ns, E]
    q_emb = l2norm(query @ W + b)     [Nq, E]
    sims  = q_emb @ s_emb.T           [Nq, Ns]
    preds = softmax(sims, axis=1) @ one_hot(labels, C)   [Nq, C]

Sharding: query rows are data-parallel (1024 per core). The support
encode is also sharded (512 rows per core) and the normalized support
embeddings are AllGathered on-chip (1 MiB/core bf16), which halves the
per-core FLOPs vs replicating the support encode on every core.

Device layout: embeddings are computed TRANSPOSED ([emb, n] with emb on
partitions) so the whole chain needs no transposes:
    s_embT tile = W_chunk.T @ supportT_chunk   (lhsT = W as stored)
    simsT  tile = s_normT_chunk.T @ q_normT    ([sup, q] layout)
    preds       = exp_simsT_chunk.T @ one_hot_aug  ([q, C+1] layout)
one_hot is augmented with a ones column so the softmax denominator
falls out of the same matmul; division happens per query partition.
Cosine sims are in [-1, 1] so softmax needs no max subtraction.

Matmul inputs are bf16 (fp32 PSUM accumulation); error << the 2e-2 gate.
"""

import numpy as np
import ml_dtypes

import concourse.bacc as bacc
import concourse.mybir as mybir
import concourse.tile as tile
from concourse.bass_utils import run_bass_kernel_spmd

F32 = mybir.dt.float32
BF16 = mybir.dt.bfloat16
AF = mybir.ActivationFunctionType

# Full-problem config (hardcoded; the grading harness provides exactly these)
N_SUPPORT = 4096
N_QUERY = 8192
IN_DIM = 2048
EMB_DIM = 1024
N_CLS = 64
N_CORES = 8
NQ_SHARD = N_QUERY // N_CORES  # 1024 query rows per core


def build_nc(NS, NQ, IN, EMB, NCLS, n_cores=N_CORES, shard_support=True):
    """Per-core Bass program. NCLS includes the +1 ones column.

    NS is the GLOBAL support count; with shard_support each core encodes
    NS/n_cores rows and AllGathers the normalized embeddings.
    """
    KCH = IN // 128    # contraction chunks for the encoder matmul
    MCH = EMB // 128   # emb chunks (partition blocks of the embT layout)
    SCH = NS // 128    # support chunks
    NS_SH = NS // n_cores if shard_support else NS
    assert NS % 512 == 0 and NQ % 512 == 0 and IN % 128 == 0 and EMB % 128 == 0
    assert (not shard_support) or NS_SH % 512 == 0

    nc = bacc.Bacc()
    supT = nc.declare_dram_parameter("supT", [IN, NS_SH], BF16, isOutput=False)
    qT = nc.declare_dram_parameter("qT", [IN, NQ], BF16, isOutput=False)
    Wd = nc.declare_dram_parameter("W", [IN, EMB], BF16, isOutput=False)
    bd = nc.declare_dram_parameter("b", [EMB], F32, isOutput=False)
    ohd = nc.declare_dram_parameter("onehot", [NS, NCLS], BF16, isOutput=False)
    outd = nc.declare_dram_parameter("out", [NQ, NCLS - 1], F32, isOutput=True)

    with tile.TileContext(nc) as tc:
        with (
            tc.tile_pool(name="singles", bufs=1) as singles,
            tc.tile_pool(name="emb_pool", bufs=1) as emb_pool,
            tc.tile_pool(name="small", bufs=4) as small,
            tc.tile_pool(name="ps_mm", bufs=3, space="PSUM") as ps_mm,
            tc.tile_pool(name="ps_n2", bufs=2, space="PSUM") as ps_n2,
            tc.tile_pool(name="ps_pred", bufs=2, space="PSUM") as ps_pred,
        ):
            # bias as [128, MCH]: b_sb[p, m] = b[m*128 + p]
            b_sb = singles.tile([128, MCH], F32)
            nc.sync.dma_start(out=b_sb, in_=bd.rearrange("(m p) -> p m", p=128))
            ones_sb = singles.tile([128, 1], BF16)
            nc.vector.memset(ones_sb, 1.0)
            # one_hot_aug chunks: oh_sb[p, c, :] = onehot[c*128 + p, :]
            # (vector queue: keep the sync queue free for the encoder loads)
            oh_sb = singles.tile([128, SCH, NCLS], BF16)
            nc.scalar.dma_start(out=oh_sb, in_=ohd.rearrange("(c p) h -> p c h", p=128))

            # resident normalized embeddings, transposed ([emb, n], bf16)
            s_nrm = emb_pool.tile([128, MCH, NS], BF16, name="s_nrm", tag="s_nrm")
            q_nrm = emb_pool.tile([128, MCH, NQ], BF16, name="q_nrm", tag="q_nrm")

            with (
                tc.tile_pool(name="w_pool", bufs=1) as w_pool,
                tc.tile_pool(name="xin", bufs=2) as xin,
                tc.tile_pool(name="pre_pool", bufs=2) as pre_pool,
                tc.tile_pool(name="sq_pool", bufs=1) as sq_pool,
                tc.tile_pool(name="bc_pool", bufs=2) as bc_pool,
                tc.tile_pool(name="loc_pool", bufs=1) as loc_pool,
                tc.tile_pool(name="dscr", bufs=2, space="DRAM") as dscr,
                tc.tile_pool(name="cc_pool", bufs=1, space="DRAM") as cc_pool,
            ):
                # W chunks, one tile per emb block m: W_sb[m][p, k, :] =
                # W[k*128 + p, m*128:(m+1)*128]. Separate DMAs so the first
                # matmul group only waits on its own 512KB slice.
                W_sb = []
                for m in range(MCH):
                    wt = w_pool.tile([128, KCH, 128], BF16, tag=f"w{m}",
                                     name=f"w{m}")
                    nc.sync.dma_start(
                        out=wt,
                        in_=Wd[:, m * 128:(m + 1) * 128]
                            .rearrange("(k p) e -> p k e", p=128))
                    W_sb.append(wt)

                def encode(xT_dram, NV, res):
                    """res[:, m, v] = l2norm(x @ W + b).T in bf16, emb-chunked."""
                    xT_r = xT_dram.rearrange("(k p) v -> p k v", p=128)
                    for nb in range(NV // 512):
                        vs = slice(nb * 512, (nb + 1) * 512)
                        xk = xin.tile([128, KCH, 512], BF16, tag="xk", name="xk")
                        nc.sync.dma_start(out=xk, in_=xT_r[:, :, vs])
                        n2 = ps_n2.tile([1, 512], F32, tag="n2", name="n2")
                        pre = pre_pool.tile([128, MCH, 512], BF16, tag="pre", name="pre")
                        sq = sq_pool.tile([128, MCH, 512], BF16, tag="sq", name="sq")
                        for m in range(MCH):
                            ps = ps_mm.tile([128, 512], F32, tag="mmps", name="ps")
                            for k in range(KCH):
                                nc.tensor.matmul(
                                    ps,
                                    lhsT=W_sb[m][:, k, :],
                                    rhs=xk[:, k, :],
                                    start=(k == 0),
                                    stop=(k == KCH - 1),
                                )
                            # bias add + fp32->bf16, PSUM->SBUF
                            nc.scalar.add(pre[:, m, :], ps, b_sb[:, m:m + 1])
                            nc.vector.tensor_mul(
                                sq[:, m, :], pre[:, m, :], pre[:, m, :])
                        # column-sums of squares via ones-matmuls (partition
                        # reduce); deferred behind the whole m-loop so the PE
                        # never waits on the ACT->DVE chain mid-stream
                        for m in range(MCH):
                            nc.tensor.matmul(
                                n2, lhsT=ones_sb, rhs=sq[:, m, :],
                                start=(m == 0), stop=(m == MCH - 1),
                            )
                        nrm = small.tile([1, 512], F32, tag="nrm", name="nrm")
                        nc.scalar.activation(nrm, n2, AF.Sqrt)
                        inv = small.tile([1, 512], F32, tag="inv", name="inv")
                        nc.vector.reciprocal(inv, nrm)
                        # partition-broadcast inv: SBUF[1,512] -> DRAM -> SBUF[128,512]
                        # (DMA only allows a zero partition step on DRAM sources)
                        iscr = dscr.tile([1, 512], F32, tag="iscr", name="iscr")
                        nc.sync.dma_start(out=iscr, in_=inv)
                        invb = bc_pool.tile([128, 512], F32, tag="invb", name="invb")
                        nc.sync.dma_start(out=invb, in_=iscr.partition_broadcast(128))
                        for m in range(MCH):
                            nc.vector.tensor_mul(res[:, m, vs], pre[:, m, :], invb)

                if shard_support:
                    # encode own support shard, then AllGather the normalized
                    # embeddings in G chunks so sims can start on early chunks
                    # while later ones are still in flight
                    G = 4
                    CW = NS_SH // G  # chunk width (support cols per core)
                    s_loc = loc_pool.tile([128, MCH, NS_SH], BF16, name="s_loc")
                    encode(supT, NS_SH, s_loc)
                    for g in range(G):
                        ag_in = cc_pool.tile([MCH * 128, CW], BF16,
                                             name=f"ag_in{g}", tag=f"ag_in{g}")
                        nc.sync.dma_start(
                            out=ag_in.rearrange("(m p) v -> p m v", p=128),
                            in_=s_loc[:, :, g * CW:(g + 1) * CW])
                        ag_out = cc_pool.tile(
                            [n_cores * MCH * 128, CW], BF16, name=f"ag_out{g}",
                            tag=f"ag_out{g}", addr_space="Shared")
                        nc.gpsimd.collective_compute(
                            "AllGather",
                            mybir.AluOpType.bypass,
                            replica_groups=[list(range(n_cores))],
                            ins=[ag_in],
                            outs=[ag_out],
                        )
                        # global support block c*G+g <- core c's chunk g
                        # (vector queue: sync is busy with encoder loads)
                        for c in range(n_cores):
                            nc.scalar.dma_start(
                                out=s_nrm[:, :, (c * G + g) * CW:(c * G + g + 1) * CW],
                                in_=ag_out[c * MCH * 128:(c + 1) * MCH * 128, :]
                                    .rearrange("(m p) v -> p m v", p=128),
                            )
                else:
                    encode(supT, NS, s_nrm)
                encode(qT, NQ, q_nrm)

            with tc.tile_pool(name="exp_pool", bufs=1) as exp_pool:
                # exp(simsT) in [sup, q] layout, bf16, sup-chunked.
                # Iterate gather-chunk-major so each AllGather chunk is
                # consumed as soon as it lands.
                if shard_support:
                    sb_order = [c * G + g for g in range(G) for c in range(n_cores)]
                else:
                    sb_order = list(range(SCH))
                expT = exp_pool.tile([128, SCH, NQ], BF16)
                for sb in sb_order:
                    ss = slice(sb * 128, (sb + 1) * 128)
                    for qh in range(NQ // 512):
                        qs = slice(qh * 512, (qh + 1) * 512)
                        ps = ps_mm.tile([128, 512], F32, tag="mmps", name="ps")
                        for m in range(MCH):
                            nc.tensor.matmul(
                                ps,
                                lhsT=s_nrm[:, m, ss],
                                rhs=q_nrm[:, m, qs],
                                start=(m == 0),
                                stop=(m == MCH - 1),
                            )
                        nc.scalar.activation(expT[:, sb, qs], ps, AF.Exp)

                with tc.tile_pool(name="outp", bufs=2) as outp:
                    for qb in range(NQ // 128):
                        qs = slice(qb * 128, (qb + 1) * 128)
                        pp = ps_pred.tile([128, NCLS], F32, tag="pp", name="pp")
                        for sb in range(SCH):
                            nc.tensor.matmul(
                                pp,
                                lhsT=expT[:, sb, qs],
                                rhs=oh_sb[:, sb, :],
                                start=(sb == 0),
                                stop=(sb == SCH - 1),
                            )
                        # softmax denominator is the ones column; divide
                        rec = small.tile([128, 1], F32, tag="rec", name="rec")
                        nc.vector.reciprocal(rec, pp[:, NCLS - 1:NCLS])
                        ot = outp.tile([128, NCLS - 1], F32, tag="ot", name="ot")
                        nc.vector.tensor_scalar_mul(ot, pp[:, 0:NCLS - 1], rec)
                        nc.sync.dma_start(out=outd[qs, :], in_=ot)
    nc.finalize()
    return nc


_NC_CACHE = {}


def _get_nc(key):
    if key not in _NC_CACHE:
        NS, NQ, IN, EMB, NCLS = key
        _NC_CACHE[key] = build_nc(NS, NQ, IN, EMB, NCLS)
    return _NC_CACHE[key]


def _prep_inputs(support, query, W, b, support_labels, num_classes, n_cores,
                 shard_support=True):
    ncls = int(num_classes)
    bf = ml_dtypes.bfloat16
    supT = np.ascontiguousarray(np.asarray(support, np.float32).T.astype(bf))
    qT = np.ascontiguousarray(np.asarray(query, np.float32).T.astype(bf))
    Wb = np.ascontiguousarray(np.asarray(W, np.float32).astype(bf))
    b32 = np.ascontiguousarray(np.asarray(b, np.float32))
    labels = np.asarray(support_labels).astype(np.int64)
    oh = np.zeros((labels.shape[0], ncls + 1), dtype=bf)
    oh[np.arange(labels.shape[0]), labels] = 1
    oh[:, ncls] = 1  # ones column -> softmax denominator
    nq_shard = qT.shape[1] // n_cores
    ns_shard = supT.shape[1] // n_cores if shard_support else supT.shape[1]
    in_maps = []
    for i in range(n_cores):
        sup_i = (np.ascontiguousarray(supT[:, i * ns_shard:(i + 1) * ns_shard])
                 if shard_support else supT)
        in_maps.append({
            "supT": sup_i,
            "qT": np.ascontiguousarray(qT[:, i * nq_shard:(i + 1) * nq_shard]),
            "W": Wb,
            "b": b32,
            "onehot": oh,
        })
    return in_maps


def _run(support, query, W, b, support_labels, num_classes, trace=False):
    ncls = int(num_classes)
    key = (support.shape[0], query.shape[0] // N_CORES, support.shape[1],
           W.shape[1], ncls + 1)
    nc = _get_nc(key)
    in_maps = _prep_inputs(support, query, W, b, support_labels, ncls, N_CORES)
    res = run_bass_kernel_spmd(nc, in_maps, list(range(N_CORES)), trace=trace)
    out = np.concatenate([r["out"] for r in res.results], axis=0)
    return out.astype(np.float32), res


def kernel(support, query, W, b, support_labels, num_classes):
    out, _ = _run(support, query, W, b, support_labels, num_classes, trace=False)
    return out


# revision 20
# speedup vs baseline: 1.1486x; 1.1486x over previous
"""MatchingNet forward on 8 Trainium2 NeuronCores (Bass/Tile).

Math (reference):
    s_emb = l2norm(support @ W + b)   [Ns, E]
    q_emb = l2norm(query @ W + b)     [Nq, E]
    sims  = q_emb @ s_emb.T           [Nq, Ns]
    preds = softmax(sims, axis=1) @ one_hot(labels, C)   [Nq, C]

Sharding: query rows are data-parallel (1024 per core). The support
encode is also sharded (512 rows per core) and the normalized support
embeddings are AllGathered on-chip in chunks (so sims starts on the
first chunk while later ones are in flight), which halves the per-core
FLOPs vs replicating the support encode on every core.

Device layout: embeddings are computed TRANSPOSED ([emb, n] with emb on
partitions) so the whole chain needs no transposes:
    s_embT tile = W_chunk.T @ supportT_chunk   (lhsT = W as stored)
    simsT  tile = s_normT_chunk.T @ q_normT    ([sup, q] layout)
    preds       = exp_simsT_chunk.T @ one_hot_aug  ([q, C+1] layout)
one_hot is augmented with a ones column so the softmax denominator
falls out of the same matmul; division happens per query partition.
Cosine sims are in [-1, 1] so softmax needs no max subtraction.

All device inputs are pre-laid-out on the host so every input DMA is a
contiguous copy. Matmul inputs are bf16 (fp32 PSUM accumulation);
error << the 2e-2 gate.

Scheduling notes (hard-won):
 - nothing that waits on the collective may sit ahead of encoder work
   in any engine's instruction stream (the engine would block on the
   collective semaphore and kill the overlap) -- so the gather-read
   DMAs are issued AFTER both encodes, on the sync queue;
 - the ones-matmuls (norm partition-reduction) are deferred behind each
   512-column block's main matmuls so the PE never waits mid-stream on
   the ACT->DVE square chain.
"""

import numpy as np
import ml_dtypes

import concourse.bacc as bacc
import concourse.mybir as mybir
import concourse.tile as tile
from concourse.bass_utils import run_bass_kernel_spmd

F32 = mybir.dt.float32
BF16 = mybir.dt.bfloat16
AF = mybir.ActivationFunctionType

# Full-problem config (hardcoded; the grading harness provides exactly these)
N_SUPPORT = 4096
N_QUERY = 8192
IN_DIM = 2048
EMB_DIM = 1024
N_CLS = 64
N_CORES = 8
NQ_SHARD = N_QUERY // N_CORES  # 1024 query rows per core
GATHER_CHUNKS = 2


def build_nc(NS, NQ, IN, EMB, NCLS, n_cores=N_CORES, shard_support=True):
    """Per-core Bass program. NCLS includes the +1 ones column.

    NS is the GLOBAL support count; with shard_support each core encodes
    NS/n_cores rows and AllGathers the normalized embeddings.
    """
    KCH = IN // 128    # contraction chunks for the encoder matmul
    MCH = EMB // 128   # emb chunks (partition blocks of the embT layout)
    SCH = NS // 128    # support chunks
    NS_SH = NS // n_cores if shard_support else NS
    NB_S = NS_SH // 512
    NB_Q = NQ // 512
    assert NS % 512 == 0 and NQ % 512 == 0 and IN % 128 == 0 and EMB % 128 == 0
    assert (not shard_support) or NS_SH % 512 == 0

    nc = bacc.Bacc()
    # host-pre-laid-out inputs (see _prep_inputs): every DMA is contiguous
    supX = nc.declare_dram_parameter("supX", [NB_S, 128, KCH, 512], BF16,
                                     isOutput=False)
    qX = nc.declare_dram_parameter("qX", [NB_Q, 128, KCH, 512], BF16,
                                   isOutput=False)
    Wd = nc.declare_dram_parameter("W", [MCH, 128, KCH, 128], BF16,
                                   isOutput=False)
    bd = nc.declare_dram_parameter("b", [128, MCH], F32, isOutput=False)
    ohd = nc.declare_dram_parameter("onehot", [128, SCH, NCLS], BF16,
                                    isOutput=False)
    outd = nc.declare_dram_parameter("out", [NQ, NCLS - 1], F32, isOutput=True)

    with tile.TileContext(nc) as tc:
        with (
            tc.tile_pool(name="singles", bufs=1) as singles,
            tc.tile_pool(name="emb_pool", bufs=1) as emb_pool,
            tc.tile_pool(name="small", bufs=4) as small,
            tc.tile_pool(name="ps_mm", bufs=3, space="PSUM") as ps_mm,
            tc.tile_pool(name="ps_n2", bufs=2, space="PSUM") as ps_n2,
            tc.tile_pool(name="ps_pred", bufs=2, space="PSUM") as ps_pred,
        ):
            b_sb = singles.tile([128, MCH], F32)
            nc.sync.dma_start(out=b_sb, in_=bd[:, :])
            ones_sb = singles.tile([128, 1], BF16)
            nc.vector.memset(ones_sb, 1.0)
            # one_hot_aug chunks on the scalar queue (sync stays free for
            # the encoder loads; nothing needs onehot until preds)
            oh_sb = singles.tile([128, SCH, NCLS], BF16)
            nc.scalar.dma_start(out=oh_sb, in_=ohd[:, :, :])

            # resident normalized embeddings, transposed ([emb, n], bf16)
            s_nrm = emb_pool.tile([128, MCH, NS], BF16, name="s_nrm", tag="s_nrm")
            q_nrm = emb_pool.tile([128, MCH, NQ], BF16, name="q_nrm", tag="q_nrm")

            with (
                tc.tile_pool(name="w_pool", bufs=1) as w_pool,
                tc.tile_pool(name="xin", bufs=2) as xin,
                tc.tile_pool(name="pre_pool", bufs=2) as pre_pool,
                tc.tile_pool(name="sq_pool", bufs=1) as sq_pool,
                tc.tile_pool(name="bc_pool", bufs=2) as bc_pool,
                tc.tile_pool(name="loc_pool", bufs=1) as loc_pool,
                tc.tile_pool(name="dscr", bufs=2, space="DRAM") as dscr,
                tc.tile_pool(name="cc_pool", bufs=1, space="DRAM") as cc_pool,
            ):
                # W tiles, one per emb block m (contiguous 512KB DMAs; the
                # first matmul group only waits on its own slice)
                W_sb = []
                for m in range(MCH):
                    wt = w_pool.tile([128, KCH, 128], BF16, tag=f"w{m}",
                                     name=f"w{m}")
                    nc.sync.dma_start(out=wt, in_=Wd[m])
                    W_sb.append(wt)

                def encode(xX, NB, res):
                    """res[:, m, v] = l2norm(x @ W + b).T in bf16, emb-chunked."""
                    for nb in range(NB):
                        vs = slice(nb * 512, (nb + 1) * 512)
                        xk = xin.tile([128, KCH, 512], BF16, tag="xk", name="xk")
                        nc.sync.dma_start(out=xk, in_=xX[nb])
                        n2 = ps_n2.tile([1, 512], F32, tag="n2", name="n2")
                        pre = pre_pool.tile([128, MCH, 512], BF16, tag="pre",
                                            name="pre")
                        sq = sq_pool.tile([128, MCH, 512], BF16, tag="sq",
                                          name="sq")
                        for m in range(MCH):
                            ps = ps_mm.tile([128, 512], F32, tag="mmps", name="ps")
                            for k in range(KCH):
                                nc.tensor.matmul(
                                    ps,
                                    lhsT=W_sb[m][:, k, :],
                                    rhs=xk[:, k, :],
                                    start=(k == 0),
                                    stop=(k == KCH - 1),
                                )
                            # bias add + fp32->bf16, PSUM->SBUF
                            nc.scalar.add(pre[:, m, :], ps, b_sb[:, m:m + 1])
                            nc.vector.tensor_mul(
                                sq[:, m, :], pre[:, m, :], pre[:, m, :])
                        # column-sums of squares via ones-matmuls (partition
                        # reduce); deferred behind the whole m-loop so the PE
                        # never waits on the ACT->DVE chain mid-stream
                        for m in range(MCH):
                            nc.tensor.matmul(
                                n2, lhsT=ones_sb, rhs=sq[:, m, :],
                                start=(m == 0), stop=(m == MCH - 1),
                            )
                        nrm = small.tile([1, 512], F32, tag="nrm", name="nrm")
                        nc.scalar.activation(nrm, n2, AF.Sqrt)
                        inv = small.tile([1, 512], F32, tag="inv", name="inv")
                        nc.vector.reciprocal(inv, nrm)
                        # partition-broadcast inv: SBUF[1,512] -> DRAM -> SBUF
                        # (DMA only allows a zero partition step on DRAM sources)
                        iscr = dscr.tile([1, 512], F32, tag="iscr", name="iscr")
                        nc.sync.dma_start(out=iscr, in_=inv)
                        invb = bc_pool.tile([128, 512], F32, tag="invb",
                                            name="invb")
                        nc.sync.dma_start(out=invb,
                                          in_=iscr.partition_broadcast(128))
                        for m in range(MCH):
                            nc.vector.tensor_mul(res[:, m, vs], pre[:, m, :], invb)

                if shard_support:
                    G = GATHER_CHUNKS
                    CW = NS_SH // G   # gathered cols per core per chunk
                    BPC = CW // 128   # 128-blocks per chunk
                    # encode own support shard, then AllGather the normalized
                    # embeddings in G chunks so sims can start on early chunks
                    s_loc = loc_pool.tile([128, MCH, NS_SH], BF16, name="s_loc")
                    encode(supX, NB_S, s_loc)
                    ag_outs = []
                    for g in range(G):
                        ag_in = cc_pool.tile([MCH * 128, CW], BF16,
                                             name=f"ag_in{g}", tag=f"ag_in{g}")
                        nc.sync.dma_start(
                            out=ag_in.rearrange("(m p) v -> p m v", p=128),
                            in_=s_loc[:, :, g * CW:(g + 1) * CW])
                        ag_out = cc_pool.tile(
                            [n_cores * MCH * 128, CW], BF16, name=f"ag_out{g}",
                            tag=f"ag_out{g}", addr_space="Shared")
                        nc.gpsimd.collective_compute(
                            "AllGather",
                            mybir.AluOpType.bypass,
                            replica_groups=[list(range(n_cores))],
                            ins=[ag_in],
                            outs=[ag_out],
                        )
                        ag_outs.append(ag_out)
                else:
                    encode(supX, NB_S, s_nrm)
                encode(qX, NB_Q, q_nrm)

                if shard_support:
                    # gather-read DMAs AFTER all encode work (they wait on the
                    # collective semaphore; anything queued behind them would
                    # stall, see module docstring). Global support block
                    # c*(NS_SH/128) + g*BPC + i  <-  core c, chunk g, block i.
                    for g in range(G):
                        for c in range(n_cores):
                            blk = c * (NS_SH // 128) + g * BPC
                            nc.sync.dma_start(
                                out=s_nrm[:, :, blk * 128:blk * 128 + CW],
                                in_=ag_outs[g][c * MCH * 128:(c + 1) * MCH * 128, :]
                                    .rearrange("(m p) v -> p m v", p=128),
                            )

            with tc.tile_pool(name="exp_pool", bufs=1) as exp_pool:
                # exp(simsT) in [sup, q] layout, bf16, sup-chunked.
                # Iterate gather-chunk-major so each AllGather chunk is
                # consumed as soon as it lands.
                if shard_support:
                    sb_order = [c * (NS_SH // 128) + g * BPC + i
                                for g in range(G)
                                for c in range(n_cores)
                                for i in range(BPC)]
                else:
                    sb_order = list(range(SCH))
                expT = exp_pool.tile([128, SCH, NQ], BF16)
                for sb in sb_order:
                    ss = slice(sb * 128, (sb + 1) * 128)
                    for qh in range(NQ // 512):
                        qs = slice(qh * 512, (qh + 1) * 512)
                        ps = ps_mm.tile([128, 512], F32, tag="mmps", name="ps")
                        for m in range(MCH):
                            nc.tensor.matmul(
                                ps,
                                lhsT=s_nrm[:, m, ss],
                                rhs=q_nrm[:, m, qs],
                                start=(m == 0),
                                stop=(m == MCH - 1),
                            )
                        nc.scalar.activation(expT[:, sb, qs], ps, AF.Exp)

                with tc.tile_pool(name="outp", bufs=2) as outp:
                    for qb in range(NQ // 128):
                        qs = slice(qb * 128, (qb + 1) * 128)
                        pp = ps_pred.tile([128, NCLS], F32, tag="pp", name="pp")
                        for sb in range(SCH):
                            nc.tensor.matmul(
                                pp,
                                lhsT=expT[:, sb, qs],
                                rhs=oh_sb[:, sb, :],
                                start=(sb == 0),
                                stop=(sb == SCH - 1),
                            )
                        # softmax denominator is the ones column; divide
                        rec = small.tile([128, 1], F32, tag="rec", name="rec")
                        nc.vector.reciprocal(rec, pp[:, NCLS - 1:NCLS])
                        ot = outp.tile([128, NCLS - 1], F32, tag="ot", name="ot")
                        nc.vector.tensor_scalar_mul(ot, pp[:, 0:NCLS - 1], rec)
                        nc.sync.dma_start(out=outd[qs, :], in_=ot)
    nc.finalize()
    return nc


_NC_CACHE = {}


def _get_nc(key):
    if key not in _NC_CACHE:
        NS, NQ, IN, EMB, NCLS = key
        _NC_CACHE[key] = build_nc(NS, NQ, IN, EMB, NCLS)
    return _NC_CACHE[key]


def _x_layout(x, kch):
    """[NV, IN] fp32 -> [NV/512, 128, KCH, 512] bf16 so each 512-row encoder
    block is one contiguous 1MiB DMA: H[nb,p,k,v] = x[nb*512+v, k*128+p]."""
    nv, in_dim = x.shape
    h = x.reshape(nv // 512, 512, kch, 128).transpose(0, 3, 2, 1)
    return np.ascontiguousarray(h.astype(ml_dtypes.bfloat16))


def _prep_inputs(support, query, W, b, support_labels, num_classes, n_cores,
                 shard_support=True):
    ncls = int(num_classes)
    bf = ml_dtypes.bfloat16
    support = np.asarray(support, np.float32)
    query = np.asarray(query, np.float32)
    W = np.asarray(W, np.float32)
    in_dim, emb = W.shape
    kch, mch = in_dim // 128, emb // 128
    ns = support.shape[0]
    # W[m, p, k, e] = W[k*128+p, m*128+e]
    Wh = np.ascontiguousarray(
        W.reshape(kch, 128, mch, 128).transpose(2, 1, 0, 3).astype(bf))
    # b[p, m] = b[m*128+p]
    bh = np.ascontiguousarray(np.asarray(b, np.float32).reshape(mch, 128).T)
    labels = np.asarray(support_labels).astype(np.int64)
    oh = np.zeros((ns, ncls + 1), dtype=bf)
    oh[np.arange(ns), labels] = 1
    oh[:, ncls] = 1  # ones column -> softmax denominator
    # oh[p, c, h] = onehot[c*128+p, h]
    ohh = np.ascontiguousarray(
        oh.reshape(ns // 128, 128, ncls + 1).transpose(1, 0, 2))
    nq_shard = query.shape[0] // n_cores
    ns_shard = ns // n_cores if shard_support else ns
    qh_all = _x_layout(query, kch)  # [NQ/512, 128, KCH, 512]
    nbq = nq_shard // 512
    in_maps = []
    for i in range(n_cores):
        sup_i = support[i * ns_shard:(i + 1) * ns_shard] if shard_support else support
        in_maps.append({
            "supX": _x_layout(sup_i, kch),
            "qX": np.ascontiguousarray(qh_all[i * nbq:(i + 1) * nbq]),
            "W": Wh,
            "b": bh,
            "onehot": ohh,
        })
    return in_maps


def _run(support, query, W, b, support_labels, num_classes, trace=False):
    ncls = int(num_classes)
    key = (support.shape[0], query.shape[0] // N_CORES, support.shape[1],
           W.shape[1], ncls + 1)
    nc = _get_nc(key)
    in_maps = _prep_inputs(support, query, W, b, support_labels, ncls, N_CORES)
    res = run_bass_kernel_spmd(nc, in_maps, list(range(N_CORES)), trace=trace)
    out = np.concatenate([r["out"] for r in res.results], axis=0)
    return out.astype(np.float32), res


def kernel(support, query, W, b, support_labels, num_classes):
    out, _ = _run(support, query, W, b, support_labels, num_classes, trace=False)
    return out


# revision 21
# speedup vs baseline: 1.2464x; 1.0851x over previous
"""MatchingNet forward on 8 Trainium2 NeuronCores (Bass/Tile).

Math (reference):
    s_emb = l2norm(support @ W + b)   [Ns, E]
    q_emb = l2norm(query @ W + b)     [Nq, E]
    sims  = q_emb @ s_emb.T           [Nq, Ns]
    preds = softmax(sims, axis=1) @ one_hot(labels, C)   [Nq, C]

Sharding: query rows are data-parallel (1024 per core). The support
encode is also sharded (512 rows per core) and the normalized support
embeddings are AllGathered on-chip in chunks (so sims starts on the
first chunk while later ones are in flight), which halves the per-core
FLOPs vs replicating the support encode on every core.

Device layout: embeddings are computed TRANSPOSED ([emb, n] with emb on
partitions) so the whole chain needs no transposes:
    s_embT tile = W_chunk.T @ supportT_chunk   (lhsT = W as stored)
    simsT  tile = s_normT_chunk.T @ q_normT    ([sup, q] layout)
    preds       = exp_simsT_chunk.T @ one_hot_aug  ([q, C+1] layout)
one_hot is augmented with a ones column so the softmax denominator
falls out of the same matmul; division happens per query partition.
Cosine sims are in [-1, 1] so softmax needs no max subtraction.

All device inputs are pre-laid-out on the host so every input DMA is a
contiguous copy. Matmul inputs are bf16 (fp32 PSUM accumulation);
error << the 2e-2 gate.

Scheduling notes (hard-won):
 - nothing that waits on the collective may sit ahead of encoder work
   in any engine's instruction stream (the engine would block on the
   collective semaphore and kill the overlap) -- so the gather-read
   DMAs are issued AFTER both encodes, on the sync queue;
 - the ones-matmuls (norm partition-reduction) are deferred behind each
   512-column block's main matmuls so the PE never waits mid-stream on
   the ACT->DVE square chain.
"""

import numpy as np
import ml_dtypes

import concourse.bacc as bacc
import concourse.mybir as mybir
import concourse.tile as tile
from concourse.bass_utils import run_bass_kernel_spmd

F32 = mybir.dt.float32
BF16 = mybir.dt.bfloat16
AF = mybir.ActivationFunctionType

# Full-problem config (hardcoded; the grading harness provides exactly these)
N_SUPPORT = 4096
N_QUERY = 8192
IN_DIM = 2048
EMB_DIM = 1024
N_CLS = 64
N_CORES = 8
NQ_SHARD = N_QUERY // N_CORES  # 1024 query rows per core
GATHER_CHUNKS = 2


def build_nc(NS, NQ, IN, EMB, NCLS, n_cores=N_CORES, shard_support=True):
    """Per-core Bass program. NCLS includes the +1 ones column.

    NS is the GLOBAL support count; with shard_support each core encodes
    NS/n_cores rows and AllGathers the normalized embeddings.
    """
    KCH = IN // 128    # contraction chunks for the encoder matmul
    MCH = EMB // 128   # emb chunks (partition blocks of the embT layout)
    SCH = NS // 128    # support chunks
    NS_SH = NS // n_cores if shard_support else NS
    NB_S = NS_SH // 512
    NB_Q = NQ // 512
    assert NS % 512 == 0 and NQ % 512 == 0 and IN % 128 == 0 and EMB % 128 == 0
    assert (not shard_support) or NS_SH % 512 == 0

    nc = bacc.Bacc()
    # host-pre-laid-out inputs (see _prep_inputs): every DMA is contiguous
    supX = nc.declare_dram_parameter("supX", [NB_S, 128, KCH, 512], BF16,
                                     isOutput=False)
    qX = nc.declare_dram_parameter("qX", [NB_Q, 128, KCH, 512], BF16,
                                   isOutput=False)
    Wd = nc.declare_dram_parameter("W", [MCH, 128, KCH, 128], BF16,
                                   isOutput=False)
    bd = nc.declare_dram_parameter("b", [128, MCH], F32, isOutput=False)
    ohd = nc.declare_dram_parameter("onehot", [128, SCH, NCLS], BF16,
                                    isOutput=False)
    outd = nc.declare_dram_parameter("out", [NQ, NCLS - 1], F32, isOutput=True)

    with tile.TileContext(nc) as tc:
        with (
            tc.tile_pool(name="singles", bufs=1) as singles,
            tc.tile_pool(name="emb_pool", bufs=1) as emb_pool,
            tc.tile_pool(name="small", bufs=4) as small,
            tc.tile_pool(name="ps_mm", bufs=3, space="PSUM") as ps_mm,
            tc.tile_pool(name="ps_n2", bufs=2, space="PSUM") as ps_n2,
            tc.tile_pool(name="ps_pred", bufs=2, space="PSUM") as ps_pred,
        ):
            b_sb = singles.tile([128, MCH], F32)
            nc.sync.dma_start(out=b_sb, in_=bd[:, :])
            ones_sb = singles.tile([128, 1], BF16)
            nc.vector.memset(ones_sb, 1.0)
            # one_hot_aug chunks on the scalar queue (sync stays free for
            # the encoder loads; nothing needs onehot until preds)
            oh_sb = singles.tile([128, SCH, NCLS], BF16)
            nc.scalar.dma_start(out=oh_sb, in_=ohd[:, :, :])

            # resident normalized embeddings, transposed ([emb, n], bf16)
            q_nrm = emb_pool.tile([128, MCH, NQ], BF16, name="q_nrm", tag="q_nrm")
            if shard_support:
                G = GATHER_CHUNKS
                CW = NS_SH // G   # gathered cols per core per chunk
                BPC = CW // 128   # 128-blocks per chunk
                # gathered support embeddings land in per-(chunk, core) tiles
                # so every gather-read DMA is contiguous on both sides
                gt = [[emb_pool.tile([128, MCH, CW], BF16, name=f"gt{g}_{c}",
                                     tag=f"gt{g}_{c}")
                       for c in range(n_cores)] for g in range(G)]
            else:
                s_nrm = emb_pool.tile([128, MCH, NS], BF16, name="s_nrm",
                                      tag="s_nrm")

            with (
                tc.tile_pool(name="w_pool", bufs=1) as w_pool,
                tc.tile_pool(name="xin", bufs=2) as xin,
                tc.tile_pool(name="pre_pool", bufs=2) as pre_pool,
                tc.tile_pool(name="sq_pool", bufs=1) as sq_pool,
                tc.tile_pool(name="bc_pool", bufs=2) as bc_pool,
                tc.tile_pool(name="loc_pool", bufs=1) as loc_pool,
                tc.tile_pool(name="dscr", bufs=2, space="DRAM") as dscr,
                tc.tile_pool(name="cc_pool", bufs=1, space="DRAM") as cc_pool,
            ):
                # W tiles, one per emb block m (contiguous 512KB DMAs).
                # Load order: W[0], first support block, then the rest -- the
                # first matmul group starts after ~1.5MiB of DMA, not 5.5MiB.
                W_sb = [w_pool.tile([128, KCH, 128], BF16, tag=f"w{m}",
                                    name=f"w{m}") for m in range(MCH)]
                nc.sync.dma_start(out=W_sb[0], in_=Wd[0])
                xk_first = xin.tile([128, KCH, 512], BF16, tag="xk", name="xk")
                nc.sync.dma_start(out=xk_first, in_=supX[0])
                for m in range(1, MCH):
                    nc.sync.dma_start(out=W_sb[m], in_=Wd[m])

                def encode(xX, NB, res, xk0=None):
                    """res[:, m, v] = l2norm(x @ W + b).T in bf16, emb-chunked."""
                    for nb in range(NB):
                        vs = slice(nb * 512, (nb + 1) * 512)
                        if nb == 0 and xk0 is not None:
                            xk = xk0
                        else:
                            xk = xin.tile([128, KCH, 512], BF16, tag="xk",
                                          name="xk")
                            nc.sync.dma_start(out=xk, in_=xX[nb])
                        n2 = ps_n2.tile([1, 512], F32, tag="n2", name="n2")
                        pre = pre_pool.tile([128, MCH, 512], BF16, tag="pre",
                                            name="pre")
                        sq = sq_pool.tile([128, MCH, 512], BF16, tag="sq",
                                          name="sq")
                        for m in range(MCH):
                            ps = ps_mm.tile([128, 512], F32, tag="mmps", name="ps")
                            for k in range(KCH):
                                nc.tensor.matmul(
                                    ps,
                                    lhsT=W_sb[m][:, k, :],
                                    rhs=xk[:, k, :],
                                    start=(k == 0),
                                    stop=(k == KCH - 1),
                                )
                            # bias add + fp32->bf16, PSUM->SBUF
                            nc.scalar.add(pre[:, m, :], ps, b_sb[:, m:m + 1])
                            nc.vector.tensor_mul(
                                sq[:, m, :], pre[:, m, :], pre[:, m, :])
                        # column-sums of squares via ones-matmuls (partition
                        # reduce); deferred behind the whole m-loop so the PE
                        # never waits on the ACT->DVE chain mid-stream
                        for m in range(MCH):
                            nc.tensor.matmul(
                                n2, lhsT=ones_sb, rhs=sq[:, m, :],
                                start=(m == 0), stop=(m == MCH - 1),
                            )
                        nrm = small.tile([1, 512], F32, tag="nrm", name="nrm")
                        nc.scalar.activation(nrm, n2, AF.Sqrt)
                        inv = small.tile([1, 512], F32, tag="inv", name="inv")
                        nc.vector.reciprocal(inv, nrm)
                        # partition-broadcast inv: SBUF[1,512] -> DRAM -> SBUF
                        # (DMA only allows a zero partition step on DRAM sources)
                        iscr = dscr.tile([1, 512], F32, tag="iscr", name="iscr")
                        nc.sync.dma_start(out=iscr, in_=inv)
                        invb = bc_pool.tile([128, 512], F32, tag="invb",
                                            name="invb")
                        nc.sync.dma_start(out=invb,
                                          in_=iscr.partition_broadcast(128))
                        for m in range(MCH):
                            nc.vector.tensor_mul(res[:, m, vs], pre[:, m, :], invb)

                if shard_support:
                    # encode own support shard, then AllGather the normalized
                    # embeddings in G chunks so sims can start on early chunks.
                    # Buffers are partition-major so both the write and the
                    # read-back DMAs are contiguous per partition.
                    s_loc = loc_pool.tile([128, MCH, NS_SH], BF16, name="s_loc")
                    encode(supX, NB_S, s_loc, xk0=xk_first)
                    ag_outs = []
                    for g in range(G):
                        ag_in = cc_pool.tile([128, MCH * CW], BF16,
                                             name=f"ag_in{g}", tag=f"ag_in{g}")
                        nc.sync.dma_start(
                            out=ag_in.rearrange("p (m v) -> p m v", m=MCH),
                            in_=s_loc[:, :, g * CW:(g + 1) * CW])
                        ag_out = cc_pool.tile(
                            [n_cores * 128, MCH * CW], BF16, name=f"ag_out{g}",
                            tag=f"ag_out{g}", addr_space="Shared")
                        nc.gpsimd.collective_compute(
                            "AllGather",
                            mybir.AluOpType.bypass,
                            replica_groups=[list(range(n_cores))],
                            ins=[ag_in],
                            outs=[ag_out],
                        )
                        ag_outs.append(ag_out)
                else:
                    encode(supX, NB_S, s_nrm, xk0=xk_first)
                encode(qX, NB_Q, q_nrm)

                if shard_support:
                    # gather-read DMAs AFTER all encode work (they wait on the
                    # collective semaphore; anything queued behind them would
                    # stall, see module docstring)
                    for g in range(G):
                        for c in range(n_cores):
                            nc.sync.dma_start(
                                out=gt[g][c],
                                in_=ag_outs[g][c * 128:(c + 1) * 128, :]
                                    .rearrange("p (m v) -> p m v", m=MCH),
                            )

            with tc.tile_pool(name="exp_pool", bufs=1) as exp_pool:
                # exp(simsT) in [sup, q] layout, bf16, sup-chunked.
                # Iterate gather-chunk-major so each AllGather chunk is
                # consumed as soon as it lands.
                if shard_support:
                    work = [(c * (NS_SH // 128) + g * BPC + i, gt[g][c], i)
                            for g in range(G)
                            for c in range(n_cores)
                            for i in range(BPC)]
                else:
                    work = [(sb, s_nrm, sb) for sb in range(SCH)]
                expT = exp_pool.tile([128, SCH, NQ], BF16)
                for sb, src_tile, i in work:
                    ss = slice(i * 128, (i + 1) * 128)
                    for qh in range(NQ // 512):
                        qs = slice(qh * 512, (qh + 1) * 512)
                        ps = ps_mm.tile([128, 512], F32, tag="mmps", name="ps")
                        for m in range(MCH):
                            nc.tensor.matmul(
                                ps,
                                lhsT=src_tile[:, m, ss],
                                rhs=q_nrm[:, m, qs],
                                start=(m == 0),
                                stop=(m == MCH - 1),
                            )
                        nc.scalar.activation(expT[:, sb, qs], ps, AF.Exp)

                with tc.tile_pool(name="outp", bufs=2) as outp:
                    for qb in range(NQ // 128):
                        qs = slice(qb * 128, (qb + 1) * 128)
                        pp = ps_pred.tile([128, NCLS], F32, tag="pp", name="pp")
                        for sb in range(SCH):
                            nc.tensor.matmul(
                                pp,
                                lhsT=expT[:, sb, qs],
                                rhs=oh_sb[:, sb, :],
                                start=(sb == 0),
                                stop=(sb == SCH - 1),
                            )
                        # softmax denominator is the ones column; divide
                        rec = small.tile([128, 1], F32, tag="rec", name="rec")
                        nc.vector.reciprocal(rec, pp[:, NCLS - 1:NCLS])
                        ot = outp.tile([128, NCLS - 1], F32, tag="ot", name="ot")
                        nc.vector.tensor_scalar_mul(ot, pp[:, 0:NCLS - 1], rec)
                        nc.sync.dma_start(out=outd[qs, :], in_=ot)
    nc.finalize()
    return nc


_NC_CACHE = {}


def _get_nc(key):
    if key not in _NC_CACHE:
        NS, NQ, IN, EMB, NCLS = key
        _NC_CACHE[key] = build_nc(NS, NQ, IN, EMB, NCLS)
    return _NC_CACHE[key]


def _x_layout(x, kch):
    """[NV, IN] fp32 -> [NV/512, 128, KCH, 512] bf16 so each 512-row encoder
    block is one contiguous 1MiB DMA: H[nb,p,k,v] = x[nb*512+v, k*128+p]."""
    nv, in_dim = x.shape
    h = x.reshape(nv // 512, 512, kch, 128).transpose(0, 3, 2, 1)
    return np.ascontiguousarray(h.astype(ml_dtypes.bfloat16))


def _prep_inputs(support, query, W, b, support_labels, num_classes, n_cores,
                 shard_support=True):
    ncls = int(num_classes)
    bf = ml_dtypes.bfloat16
    support = np.asarray(support, np.float32)
    query = np.asarray(query, np.float32)
    W = np.asarray(W, np.float32)
    in_dim, emb = W.shape
    kch, mch = in_dim // 128, emb // 128
    ns = support.shape[0]
    # W[m, p, k, e] = W[k*128+p, m*128+e]
    Wh = np.ascontiguousarray(
        W.reshape(kch, 128, mch, 128).transpose(2, 1, 0, 3).astype(bf))
    # b[p, m] = b[m*128+p]
    bh = np.ascontiguousarray(np.asarray(b, np.float32).reshape(mch, 128).T)
    labels = np.asarray(support_labels).astype(np.int64)
    oh = np.zeros((ns, ncls + 1), dtype=bf)
    oh[np.arange(ns), labels] = 1
    oh[:, ncls] = 1  # ones column -> softmax denominator
    # oh[p, c, h] = onehot[c*128+p, h]
    ohh = np.ascontiguousarray(
        oh.reshape(ns // 128, 128, ncls + 1).transpose(1, 0, 2))
    nq_shard = query.shape[0] // n_cores
    ns_shard = ns // n_cores if shard_support else ns
    qh_all = _x_layout(query, kch)  # [NQ/512, 128, KCH, 512]
    nbq = nq_shard // 512
    in_maps = []
    for i in range(n_cores):
        sup_i = support[i * ns_shard:(i + 1) * ns_shard] if shard_support else support
        in_maps.append({
            "supX": _x_layout(sup_i, kch),
            "qX": np.ascontiguousarray(qh_all[i * nbq:(i + 1) * nbq]),
            "W": Wh,
            "b": bh,
            "onehot": ohh,
        })
    return in_maps


def _run(support, query, W, b, support_labels, num_classes, trace=False):
    ncls = int(num_classes)
    key = (support.shape[0], query.shape[0] // N_CORES, support.shape[1],
           W.shape[1], ncls + 1)
    nc = _get_nc(key)
    in_maps = _prep_inputs(support, query, W, b, support_labels, ncls, N_CORES)
    res = run_bass_kernel_spmd(nc, in_maps, list(range(N_CORES)), trace=trace)
    out = np.concatenate([r["out"] for r in res.results], axis=0)
    return out.astype(np.float32), res


def kernel(support, query, W, b, support_labels, num_classes):
    out, _ = _run(support, query, W, b, support_labels, num_classes, trace=False)
    return out


# revision 22
# speedup vs baseline: 1.3089x; 1.0502x over previous
"""MatchingNet forward on 8 Trainium2 NeuronCores (Bass/Tile).

Math (reference):
    s_emb = l2norm(support @ W + b)   [Ns, E]
    q_emb = l2norm(query @ W + b)     [Nq, E]
    sims  = q_emb @ s_emb.T           [Nq, Ns]
    preds = softmax(sims, axis=1) @ one_hot(labels, C)   [Nq, C]

Sharding: query rows are data-parallel (1024 per core). The support
encode is also sharded (512 rows per core) and the normalized support
embeddings are AllGathered on-chip in chunks (so sims starts on the
first chunk while later ones are in flight), which halves the per-core
FLOPs vs replicating the support encode on every core.

Device layout: embeddings are computed TRANSPOSED ([emb, n] with emb on
partitions) so the whole chain needs no transposes:
    s_embT tile = W_chunk.T @ supportT_chunk   (lhsT = W as stored)
    simsT  tile = s_normT_chunk.T @ q_normT    ([sup, q] layout)
    preds       = exp_simsT_chunk.T @ one_hot_aug  ([q, C+1] layout)
one_hot is augmented with a ones column so the softmax denominator
falls out of the same matmul; division happens per query partition.
Cosine sims are in [-1, 1] so softmax needs no max subtraction.

All device inputs are pre-laid-out on the host so every input DMA is a
contiguous copy. Matmul inputs are bf16 (fp32 PSUM accumulation);
error << the 2e-2 gate.

Scheduling notes (hard-won):
 - nothing that waits on the collective may sit ahead of encoder work
   in any engine's instruction stream (the engine would block on the
   collective semaphore and kill the overlap) -- so the gather-read
   DMAs are issued AFTER both encodes, on the sync queue;
 - the ones-matmuls (norm partition-reduction) are deferred behind each
   512-column block's main matmuls so the PE never waits mid-stream on
   the ACT->DVE square chain.
"""

import numpy as np
import ml_dtypes

import concourse.bacc as bacc
import concourse.mybir as mybir
import concourse.tile as tile
from concourse.bass_utils import run_bass_kernel_spmd

F32 = mybir.dt.float32
BF16 = mybir.dt.bfloat16
AF = mybir.ActivationFunctionType

# Full-problem config (hardcoded; the grading harness provides exactly these)
N_SUPPORT = 4096
N_QUERY = 8192
IN_DIM = 2048
EMB_DIM = 1024
N_CLS = 64
N_CORES = 8
NQ_SHARD = N_QUERY // N_CORES  # 1024 query rows per core
GATHER_CHUNKS = 2


def build_nc(NS, NQ, IN, EMB, NCLS, n_cores=N_CORES, shard_support=True):
    """Per-core Bass program. NCLS includes the +1 ones column.

    NS is the GLOBAL support count; with shard_support each core encodes
    NS/n_cores rows and AllGathers the normalized embeddings.
    """
    KCH = IN // 128    # contraction chunks for the encoder matmul
    MCH = EMB // 128   # emb chunks (partition blocks of the embT layout)
    SCH = NS // 128    # support chunks
    NS_SH = NS // n_cores if shard_support else NS
    NB_S = NS_SH // 512
    NB_Q = NQ // 512
    assert NS % 512 == 0 and NQ % 512 == 0 and IN % 128 == 0 and EMB % 128 == 0
    assert (not shard_support) or NS_SH % 512 == 0

    nc = bacc.Bacc()
    # host-pre-laid-out inputs (see _prep_inputs): every DMA is contiguous
    supX = nc.declare_dram_parameter("supX", [NB_S, 128, KCH, 512], BF16,
                                     isOutput=False)
    qX = nc.declare_dram_parameter("qX", [NB_Q, 128, KCH, 512], BF16,
                                   isOutput=False)
    Wd = nc.declare_dram_parameter("W", [MCH, 128, KCH, 128], BF16,
                                   isOutput=False)
    bd = nc.declare_dram_parameter("b", [128, MCH], F32, isOutput=False)
    ohd = nc.declare_dram_parameter("onehot", [128, SCH, NCLS], BF16,
                                    isOutput=False)
    outd = nc.declare_dram_parameter("out", [NQ, NCLS - 1], F32, isOutput=True)

    with tile.TileContext(nc) as tc:
        with (
            tc.tile_pool(name="singles", bufs=1) as singles,
            tc.tile_pool(name="emb_pool", bufs=1) as emb_pool,
            tc.tile_pool(name="small", bufs=4) as small,
            tc.tile_pool(name="ps_mm", bufs=3, space="PSUM") as ps_mm,
            tc.tile_pool(name="ps_n2", bufs=2, space="PSUM") as ps_n2,
            tc.tile_pool(name="ps_pred", bufs=2, space="PSUM") as ps_pred,
        ):
            b_sb = singles.tile([128, MCH], F32)
            nc.sync.dma_start(out=b_sb, in_=bd[:, :])
            ones_sb = singles.tile([128, 1], BF16)
            nc.vector.memset(ones_sb, 1.0)
            # one_hot_aug chunks on the scalar queue (sync stays free for
            # the encoder loads; nothing needs onehot until preds)
            oh_sb = singles.tile([128, SCH, NCLS], BF16)
            nc.scalar.dma_start(out=oh_sb, in_=ohd[:, :, :])

            # resident normalized embeddings, transposed ([emb, n], bf16)
            q_nrm = emb_pool.tile([128, MCH, NQ], BF16, name="q_nrm", tag="q_nrm")
            if shard_support:
                G = GATHER_CHUNKS
                CW = NS_SH // G   # gathered cols per core per chunk
                BPC = CW // 128   # 128-blocks per chunk
                # gathered support embeddings land in per-(chunk, core) tiles
                # so every gather-read DMA is contiguous on both sides
                gt = [[emb_pool.tile([128, MCH, CW], BF16, name=f"gt{g}_{c}",
                                     tag=f"gt{g}_{c}")
                       for c in range(n_cores)] for g in range(G)]
            else:
                s_nrm = emb_pool.tile([128, MCH, NS], BF16, name="s_nrm",
                                      tag="s_nrm")

            with (
                tc.tile_pool(name="w_pool", bufs=1) as w_pool,
                tc.tile_pool(name="xin", bufs=2) as xin,
                tc.tile_pool(name="pre_pool", bufs=2) as pre_pool,
                tc.tile_pool(name="sq_pool", bufs=1) as sq_pool,
                tc.tile_pool(name="bc_pool", bufs=2) as bc_pool,
                tc.tile_pool(name="loc_pool", bufs=1) as loc_pool,
                tc.tile_pool(name="dscr", bufs=2, space="DRAM") as dscr,
                tc.tile_pool(name="cc_pool", bufs=1, space="DRAM") as cc_pool,
            ):
                # W tiles, one per emb block m (contiguous 512KB DMAs).
                # Load order: W[0], first support block, then the rest -- the
                # first matmul group starts after ~1.5MiB of DMA, not 5.5MiB.
                W_sb = [w_pool.tile([128, KCH, 128], BF16, tag=f"w{m}",
                                    name=f"w{m}") for m in range(MCH)]
                nc.sync.dma_start(out=W_sb[0], in_=Wd[0])
                xk_first = xin.tile([128, KCH, 512], BF16, tag="xk", name="xk")
                nc.sync.dma_start(out=xk_first, in_=supX[0])
                for m in range(1, MCH):
                    nc.sync.dma_start(out=W_sb[m], in_=Wd[m])

                def encode(xX, NB, res, xk0=None):
                    """res[:, m, v] = l2norm(x @ W + b).T in bf16, emb-chunked."""
                    for nb in range(NB):
                        vs = slice(nb * 512, (nb + 1) * 512)
                        if nb == 0 and xk0 is not None:
                            xk = xk0
                        else:
                            xk = xin.tile([128, KCH, 512], BF16, tag="xk",
                                          name="xk")
                            nc.sync.dma_start(out=xk, in_=xX[nb])
                        n2 = ps_n2.tile([1, 512], F32, tag="n2", name="n2")
                        pre = pre_pool.tile([128, MCH, 512], BF16, tag="pre",
                                            name="pre")
                        sq = sq_pool.tile([128, MCH, 512], BF16, tag="sq",
                                          name="sq")
                        for m in range(MCH):
                            ps = ps_mm.tile([128, 512], F32, tag="mmps", name="ps")
                            for k in range(KCH):
                                nc.tensor.matmul(
                                    ps,
                                    lhsT=W_sb[m][:, k, :],
                                    rhs=xk[:, k, :],
                                    start=(k == 0),
                                    stop=(k == KCH - 1),
                                )
                            # bias add + fp32->bf16, PSUM->SBUF
                            nc.scalar.add(pre[:, m, :], ps, b_sb[:, m:m + 1])
                            nc.vector.tensor_mul(
                                sq[:, m, :], pre[:, m, :], pre[:, m, :])
                        # column-sums of squares via ones-matmuls (partition
                        # reduce); deferred behind the whole m-loop so the PE
                        # never waits on the ACT->DVE chain mid-stream
                        for m in range(MCH):
                            nc.tensor.matmul(
                                n2, lhsT=ones_sb, rhs=sq[:, m, :],
                                start=(m == 0), stop=(m == MCH - 1),
                            )
                        nrm = small.tile([1, 512], F32, tag="nrm", name="nrm")
                        nc.scalar.activation(nrm, n2, AF.Sqrt)
                        inv = small.tile([1, 512], F32, tag="inv", name="inv")
                        nc.vector.reciprocal(inv, nrm)
                        # partition-broadcast inv: SBUF[1,512] -> DRAM -> SBUF
                        # (DMA only allows a zero partition step on DRAM sources)
                        iscr = dscr.tile([1, 512], F32, tag="iscr", name="iscr")
                        nc.sync.dma_start(out=iscr, in_=inv)
                        invb = bc_pool.tile([128, 512], F32, tag="invb",
                                            name="invb")
                        nc.sync.dma_start(out=invb,
                                          in_=iscr.partition_broadcast(128))
                        for m in range(MCH):
                            nc.vector.tensor_mul(res[:, m, vs], pre[:, m, :], invb)

                if shard_support:
                    # encode own support shard, then AllGather the normalized
                    # embeddings in G chunks so sims can start on early chunks.
                    # Buffers are partition-major so both the write and the
                    # read-back DMAs are contiguous per partition.
                    s_loc = loc_pool.tile([128, MCH, NS_SH], BF16, name="s_loc")
                    encode(supX, NB_S, s_loc, xk0=xk_first)
                    ag_outs = []
                    for g in range(G):
                        ag_in = cc_pool.tile([128, MCH * CW], BF16,
                                             name=f"ag_in{g}", tag=f"ag_in{g}")
                        nc.sync.dma_start(
                            out=ag_in.rearrange("p (m v) -> p m v", m=MCH),
                            in_=s_loc[:, :, g * CW:(g + 1) * CW])
                        ag_out = cc_pool.tile(
                            [n_cores * 128, MCH * CW], BF16, name=f"ag_out{g}",
                            tag=f"ag_out{g}", addr_space="Shared")
                        nc.gpsimd.collective_compute(
                            "AllGather",
                            mybir.AluOpType.bypass,
                            replica_groups=[list(range(n_cores))],
                            ins=[ag_in],
                            outs=[ag_out],
                        )
                        ag_outs.append(ag_out)
                else:
                    encode(supX, NB_S, s_nrm, xk0=xk_first)
                encode(qX, NB_Q, q_nrm)

                if shard_support:
                    # gather-read DMAs AFTER all encode work (they wait on the
                    # collective semaphore; anything queued behind them would
                    # stall). Chunk g=0 on the sync queues, later chunks on
                    # gpsimd's queues -- on a shared queue a later chunk's
                    # collective-wait would block an earlier chunk's reads.
                    for g in range(G):
                        eng = nc.sync if g == 0 else nc.gpsimd
                        for c in range(n_cores):
                            eng.dma_start(
                                out=gt[g][c],
                                in_=ag_outs[g][c * 128:(c + 1) * 128, :]
                                    .rearrange("p (m v) -> p m v", m=MCH),
                            )

            with tc.tile_pool(name="exp_pool", bufs=1) as exp_pool:
                # exp(simsT) in [sup, q] layout, bf16, sup-chunked.
                # Iterate gather-chunk-major so each AllGather chunk is
                # consumed as soon as it lands.
                if shard_support:
                    work = [(c * (NS_SH // 128) + g * BPC + i, gt[g][c], i)
                            for g in range(G)
                            for c in range(n_cores)
                            for i in range(BPC)]
                else:
                    work = [(sb, s_nrm, sb) for sb in range(SCH)]
                expT = exp_pool.tile([128, SCH, NQ], BF16)
                for sb, src_tile, i in work:
                    ss = slice(i * 128, (i + 1) * 128)
                    for qh in range(NQ // 512):
                        qs = slice(qh * 512, (qh + 1) * 512)
                        ps = ps_mm.tile([128, 512], F32, tag="mmps", name="ps")
                        for m in range(MCH):
                            nc.tensor.matmul(
                                ps,
                                lhsT=src_tile[:, m, ss],
                                rhs=q_nrm[:, m, qs],
                                start=(m == 0),
                                stop=(m == MCH - 1),
                            )
                        nc.scalar.activation(expT[:, sb, qs], ps, AF.Exp)

                with tc.tile_pool(name="outp", bufs=2) as outp:
                    for qb in range(NQ // 128):
                        qs = slice(qb * 128, (qb + 1) * 128)
                        pp = ps_pred.tile([128, NCLS], F32, tag="pp", name="pp")
                        for sb in range(SCH):
                            nc.tensor.matmul(
                                pp,
                                lhsT=expT[:, sb, qs],
                                rhs=oh_sb[:, sb, :],
                                start=(sb == 0),
                                stop=(sb == SCH - 1),
                            )
                        # softmax denominator is the ones column; divide
                        rec = small.tile([128, 1], F32, tag="rec", name="rec")
                        nc.vector.reciprocal(rec, pp[:, NCLS - 1:NCLS])
                        ot = outp.tile([128, NCLS - 1], F32, tag="ot", name="ot")
                        nc.vector.tensor_scalar_mul(ot, pp[:, 0:NCLS - 1], rec)
                        nc.sync.dma_start(out=outd[qs, :], in_=ot)
    nc.finalize()
    return nc


_NC_CACHE = {}


def _get_nc(key):
    if key not in _NC_CACHE:
        NS, NQ, IN, EMB, NCLS = key
        _NC_CACHE[key] = build_nc(NS, NQ, IN, EMB, NCLS)
    return _NC_CACHE[key]


def _x_layout(x, kch):
    """[NV, IN] fp32 -> [NV/512, 128, KCH, 512] bf16 so each 512-row encoder
    block is one contiguous 1MiB DMA: H[nb,p,k,v] = x[nb*512+v, k*128+p]."""
    nv, in_dim = x.shape
    h = x.reshape(nv // 512, 512, kch, 128).transpose(0, 3, 2, 1)
    return np.ascontiguousarray(h.astype(ml_dtypes.bfloat16))


def _prep_inputs(support, query, W, b, support_labels, num_classes, n_cores,
                 shard_support=True):
    ncls = int(num_classes)
    bf = ml_dtypes.bfloat16
    support = np.asarray(support, np.float32)
    query = np.asarray(query, np.float32)
    W = np.asarray(W, np.float32)
    in_dim, emb = W.shape
    kch, mch = in_dim // 128, emb // 128
    ns = support.shape[0]
    # W[m, p, k, e] = W[k*128+p, m*128+e]
    Wh = np.ascontiguousarray(
        W.reshape(kch, 128, mch, 128).transpose(2, 1, 0, 3).astype(bf))
    # b[p, m] = b[m*128+p]
    bh = np.ascontiguousarray(np.asarray(b, np.float32).reshape(mch, 128).T)
    labels = np.asarray(support_labels).astype(np.int64)
    oh = np.zeros((ns, ncls + 1), dtype=bf)
    oh[np.arange(ns), labels] = 1
    oh[:, ncls] = 1  # ones column -> softmax denominator
    # oh[p, c, h] = onehot[c*128+p, h]
    ohh = np.ascontiguousarray(
        oh.reshape(ns // 128, 128, ncls + 1).transpose(1, 0, 2))
    nq_shard = query.shape[0] // n_cores
    ns_shard = ns // n_cores if shard_support else ns
    qh_all = _x_layout(query, kch)  # [NQ/512, 128, KCH, 512]
    nbq = nq_shard // 512
    in_maps = []
    for i in range(n_cores):
        sup_i = support[i * ns_shard:(i + 1) * ns_shard] if shard_support else support
        in_maps.append({
            "supX": _x_layout(sup_i, kch),
            "qX": np.ascontiguousarray(qh_all[i * nbq:(i + 1) * nbq]),
            "W": Wh,
            "b": bh,
            "onehot": ohh,
        })
    return in_maps


def _run(support, query, W, b, support_labels, num_classes, trace=False):
    ncls = int(num_classes)
    key = (support.shape[0], query.shape[0] // N_CORES, support.shape[1],
           W.shape[1], ncls + 1)
    nc = _get_nc(key)
    in_maps = _prep_inputs(support, query, W, b, support_labels, ncls, N_CORES)
    res = run_bass_kernel_spmd(nc, in_maps, list(range(N_CORES)), trace=trace)
    out = np.concatenate([r["out"] for r in res.results], axis=0)
    return out.astype(np.float32), res


def kernel(support, query, W, b, support_labels, num_classes):
    out, _ = _run(support, query, W, b, support_labels, num_classes, trace=False)
    return out


# revision 23
# speedup vs baseline: 1.4047x; 1.0732x over previous
"""MatchingNet forward on 8 Trainium2 NeuronCores (Bass/Tile).

Math (reference):
    s_emb = l2norm(support @ W + b)   [Ns, E]
    q_emb = l2norm(query @ W + b)     [Nq, E]
    sims  = q_emb @ s_emb.T           [Nq, Ns]
    preds = softmax(sims, axis=1) @ one_hot(labels, C)   [Nq, C]

Sharding: query rows are data-parallel (1024 per core). The support
encode is also sharded (512 rows per core) and the normalized support
embeddings are AllGathered on-chip in chunks (so sims starts on the
first chunk while later ones are in flight), which halves the per-core
FLOPs vs replicating the support encode on every core.

Device layout: embeddings are computed TRANSPOSED ([emb, n] with emb on
partitions) so the whole chain needs no transposes:
    s_embT tile = W_chunk.T @ supportT_chunk   (lhsT = W as stored)
    simsT  tile = s_normT_chunk.T @ q_normT    ([sup, q] layout)
    preds       = exp_simsT_chunk.T @ one_hot_aug  ([q, C+1] layout)
one_hot is augmented with a ones column so the softmax denominator
falls out of the same matmul; division happens per query partition.
Cosine sims are in [-1, 1] so softmax needs no max subtraction.

All device inputs are pre-laid-out on the host so every input DMA is a
contiguous copy. Matmul inputs are bf16 (fp32 PSUM accumulation);
error << the 2e-2 gate.

Scheduling notes (hard-won):
 - nothing that waits on the collective may sit ahead of encoder work
   in any engine's instruction stream (the engine would block on the
   collective semaphore and kill the overlap) -- so the gather-read
   DMAs are issued AFTER both encodes, on the sync queue;
 - the ones-matmuls (norm partition-reduction) are deferred behind each
   512-column block's main matmuls so the PE never waits mid-stream on
   the ACT->DVE square chain.
"""

import numpy as np
import ml_dtypes

import concourse.bacc as bacc
import concourse.mybir as mybir
import concourse.tile as tile
from concourse.bass_utils import run_bass_kernel_spmd

F32 = mybir.dt.float32
BF16 = mybir.dt.bfloat16
AF = mybir.ActivationFunctionType

# Full-problem config (hardcoded; the grading harness provides exactly these)
N_SUPPORT = 4096
N_QUERY = 8192
IN_DIM = 2048
EMB_DIM = 1024
N_CLS = 64
N_CORES = 8
NQ_SHARD = N_QUERY // N_CORES  # 1024 query rows per core
GATHER_CHUNKS = 2


def build_nc(NS, NQ, IN, EMB, NCLS, n_cores=N_CORES, shard_support=True):
    """Per-core Bass program. NCLS includes the +1 ones column.

    NS is the GLOBAL support count; with shard_support each core encodes
    NS/n_cores rows and AllGathers the normalized embeddings.
    """
    KCH = IN // 128    # contraction chunks for the encoder matmul
    MCH = EMB // 128   # emb chunks (partition blocks of the embT layout)
    SCH = NS // 128    # support chunks
    NS_SH = NS // n_cores if shard_support else NS
    NB_S = NS_SH // 512
    NB_Q = NQ // 512
    assert NS % 512 == 0 and NQ % 512 == 0 and IN % 128 == 0 and EMB % 128 == 0
    assert (not shard_support) or NS_SH % 512 == 0

    nc = bacc.Bacc()
    # host-pre-laid-out inputs (see _prep_inputs): every DMA is contiguous
    supX = nc.declare_dram_parameter("supX", [NB_S, 128, KCH, 512], BF16,
                                     isOutput=False)
    qX = nc.declare_dram_parameter("qX", [NB_Q, 128, KCH, 512], BF16,
                                   isOutput=False)
    Wd = nc.declare_dram_parameter("W", [MCH, 128, KCH, 128], BF16,
                                   isOutput=False)
    bd = nc.declare_dram_parameter("b", [128, MCH], F32, isOutput=False)
    ohd = nc.declare_dram_parameter("onehot", [128, SCH, NCLS], BF16,
                                    isOutput=False)
    outd = nc.declare_dram_parameter("out", [NQ, NCLS - 1], F32, isOutput=True)

    with tile.TileContext(nc) as tc:
        with (
            tc.tile_pool(name="singles", bufs=1) as singles,
            tc.tile_pool(name="emb_pool", bufs=1) as emb_pool,
            tc.tile_pool(name="small", bufs=4) as small,
            tc.tile_pool(name="ps_mm", bufs=3, space="PSUM") as ps_mm,
            tc.tile_pool(name="ps_n2", bufs=2, space="PSUM") as ps_n2,
            tc.tile_pool(name="ps_pred", bufs=2, space="PSUM") as ps_pred,
        ):
            b_sb = singles.tile([128, MCH], F32)
            nc.sync.dma_start(out=b_sb, in_=bd[:, :])
            ones_sb = singles.tile([128, 1], BF16)
            nc.vector.memset(ones_sb, 1.0)
            # one_hot_aug chunks on the scalar queue (sync stays free for
            # the encoder loads; nothing needs onehot until preds)
            oh_sb = singles.tile([128, SCH, NCLS], BF16)
            nc.scalar.dma_start(out=oh_sb, in_=ohd[:, :, :])

            # resident normalized embeddings, transposed ([emb, n], bf16)
            q_nrm = emb_pool.tile([128, MCH, NQ], BF16, name="q_nrm", tag="q_nrm")
            if shard_support:
                # asymmetric gather chunks: a small first chunk lands early so
                # sims can start right when the query encode frees the PE; the
                # big second chunk arrives while the first is being consumed
                CWS = [NS_SH // 4, NS_SH - NS_SH // 4]
                OFFS = [0, NS_SH // 4]
                G = len(CWS)
                # gathered support embeddings land in per-(chunk, core) tiles
                # so every gather-read DMA is contiguous on both sides
                gt = [[emb_pool.tile([128, MCH, CWS[g]], BF16,
                                     name=f"gt{g}_{c}", tag=f"gt{g}_{c}")
                       for c in range(n_cores)] for g in range(G)]
            else:
                s_nrm = emb_pool.tile([128, MCH, NS], BF16, name="s_nrm",
                                      tag="s_nrm")

            with (
                tc.tile_pool(name="w_pool", bufs=1) as w_pool,
                tc.tile_pool(name="xin", bufs=2) as xin,
                tc.tile_pool(name="pre_pool", bufs=2) as pre_pool,
                tc.tile_pool(name="sq_pool", bufs=1) as sq_pool,
                tc.tile_pool(name="bc_pool", bufs=2) as bc_pool,
                tc.tile_pool(name="loc_pool", bufs=1) as loc_pool,
                tc.tile_pool(name="dscr", bufs=2, space="DRAM") as dscr,
                tc.tile_pool(name="cc_pool", bufs=1, space="DRAM") as cc_pool,
            ):
                # W tiles, one per emb block m (contiguous 512KB DMAs).
                # Load order: W[0], first support block, then the rest -- the
                # first matmul group starts after ~1.5MiB of DMA, not 5.5MiB.
                W_sb = [w_pool.tile([128, KCH, 128], BF16, tag=f"w{m}",
                                    name=f"w{m}") for m in range(MCH)]
                nc.sync.dma_start(out=W_sb[0], in_=Wd[0])
                xk_first = xin.tile([128, KCH, 512], BF16, tag="xk", name="xk")
                nc.sync.dma_start(out=xk_first, in_=supX[0])
                for m in range(1, MCH):
                    nc.sync.dma_start(out=W_sb[m], in_=Wd[m])

                def encode(xX, NB, res, xk0=None):
                    """res[:, m, v] = l2norm(x @ W + b).T in bf16, emb-chunked."""
                    for nb in range(NB):
                        vs = slice(nb * 512, (nb + 1) * 512)
                        if nb == 0 and xk0 is not None:
                            xk = xk0
                        else:
                            xk = xin.tile([128, KCH, 512], BF16, tag="xk",
                                          name="xk")
                            nc.sync.dma_start(out=xk, in_=xX[nb])
                        n2 = ps_n2.tile([1, 512], F32, tag="n2", name="n2")
                        pre = pre_pool.tile([128, MCH, 512], BF16, tag="pre",
                                            name="pre")
                        sq = sq_pool.tile([128, MCH, 512], BF16, tag="sq",
                                          name="sq")
                        for m in range(MCH):
                            ps = ps_mm.tile([128, 512], F32, tag="mmps", name="ps")
                            for k in range(KCH):
                                nc.tensor.matmul(
                                    ps,
                                    lhsT=W_sb[m][:, k, :],
                                    rhs=xk[:, k, :],
                                    start=(k == 0),
                                    stop=(k == KCH - 1),
                                )
                            # bias add + fp32->bf16, PSUM->SBUF
                            nc.scalar.add(pre[:, m, :], ps, b_sb[:, m:m + 1])
                            nc.vector.tensor_mul(
                                sq[:, m, :], pre[:, m, :], pre[:, m, :])
                        # column-sums of squares via ones-matmuls (partition
                        # reduce); deferred behind the whole m-loop so the PE
                        # never waits on the ACT->DVE chain mid-stream
                        for m in range(MCH):
                            nc.tensor.matmul(
                                n2, lhsT=ones_sb, rhs=sq[:, m, :],
                                start=(m == 0), stop=(m == MCH - 1),
                            )
                        nrm = small.tile([1, 512], F32, tag="nrm", name="nrm")
                        nc.scalar.activation(nrm, n2, AF.Sqrt)
                        inv = small.tile([1, 512], F32, tag="inv", name="inv")
                        nc.vector.reciprocal(inv, nrm)
                        # partition-broadcast inv: SBUF[1,512] -> DRAM -> SBUF
                        # (DMA only allows a zero partition step on DRAM sources)
                        iscr = dscr.tile([1, 512], F32, tag="iscr", name="iscr")
                        nc.sync.dma_start(out=iscr, in_=inv)
                        invb = bc_pool.tile([128, 512], F32, tag="invb",
                                            name="invb")
                        nc.sync.dma_start(out=invb,
                                          in_=iscr.partition_broadcast(128))
                        for m in range(MCH):
                            nc.vector.tensor_mul(res[:, m, vs], pre[:, m, :], invb)

                if shard_support:
                    # encode own support shard, then AllGather the normalized
                    # embeddings in G chunks so sims can start on early chunks.
                    # Buffers are partition-major so both the write and the
                    # read-back DMAs are contiguous per partition.
                    s_loc = loc_pool.tile([128, MCH, NS_SH], BF16, name="s_loc")
                    encode(supX, NB_S, s_loc, xk0=xk_first)
                    ag_outs = []
                    for g in range(G):
                        CW = CWS[g]
                        ag_in = cc_pool.tile([128, MCH * CW], BF16,
                                             name=f"ag_in{g}", tag=f"ag_in{g}")
                        nc.sync.dma_start(
                            out=ag_in.rearrange("p (m v) -> p m v", m=MCH),
                            in_=s_loc[:, :, OFFS[g]:OFFS[g] + CW])
                        ag_out = cc_pool.tile(
                            [n_cores * 128, MCH * CW], BF16, name=f"ag_out{g}",
                            tag=f"ag_out{g}", addr_space="Shared")
                        nc.gpsimd.collective_compute(
                            "AllGather",
                            mybir.AluOpType.bypass,
                            replica_groups=[list(range(n_cores))],
                            ins=[ag_in],
                            outs=[ag_out],
                        )
                        ag_outs.append(ag_out)
                else:
                    encode(supX, NB_S, s_nrm, xk0=xk_first)
                encode(qX, NB_Q, q_nrm)

                if shard_support:
                    # gather-read DMAs AFTER all encode work (they wait on the
                    # collective semaphore; anything queued behind them would
                    # stall). Chunk g=0 on the sync queues, later chunks on
                    # gpsimd's queues -- on a shared queue a later chunk's
                    # collective-wait would block an earlier chunk's reads.
                    for g in range(G):
                        eng = nc.sync if g == 0 else nc.gpsimd
                        for c in range(n_cores):
                            eng.dma_start(
                                out=gt[g][c],
                                in_=ag_outs[g][c * 128:(c + 1) * 128, :]
                                    .rearrange("p (m v) -> p m v", m=MCH),
                            )

            with tc.tile_pool(name="exp_pool", bufs=1) as exp_pool:
                # exp(simsT) in [sup, q] layout, bf16, sup-chunked.
                # Iterate gather-chunk-major so each AllGather chunk is
                # consumed as soon as it lands.
                if shard_support:
                    work = [(c * (NS_SH // 128) + OFFS[g] // 128 + i, gt[g][c], i)
                            for g in range(G)
                            for c in range(n_cores)
                            for i in range(CWS[g] // 128)]
                else:
                    work = [(sb, s_nrm, sb) for sb in range(SCH)]
                expT = exp_pool.tile([128, SCH, NQ], BF16)
                for sb, src_tile, i in work:
                    ss = slice(i * 128, (i + 1) * 128)
                    for qh in range(NQ // 512):
                        qs = slice(qh * 512, (qh + 1) * 512)
                        ps = ps_mm.tile([128, 512], F32, tag="mmps", name="ps")
                        for m in range(MCH):
                            nc.tensor.matmul(
                                ps,
                                lhsT=src_tile[:, m, ss],
                                rhs=q_nrm[:, m, qs],
                                start=(m == 0),
                                stop=(m == MCH - 1),
                            )
                        nc.scalar.activation(expT[:, sb, qs], ps, AF.Exp)

                with tc.tile_pool(name="outp", bufs=2) as outp:
                    for qb in range(NQ // 128):
                        qs = slice(qb * 128, (qb + 1) * 128)
                        pp = ps_pred.tile([128, NCLS], F32, tag="pp", name="pp")
                        for sb in range(SCH):
                            nc.tensor.matmul(
                                pp,
                                lhsT=expT[:, sb, qs],
                                rhs=oh_sb[:, sb, :],
                                start=(sb == 0),
                                stop=(sb == SCH - 1),
                            )
                        # softmax denominator is the ones column; divide
                        rec = small.tile([128, 1], F32, tag="rec", name="rec")
                        nc.vector.reciprocal(rec, pp[:, NCLS - 1:NCLS])
                        ot = outp.tile([128, NCLS - 1], F32, tag="ot", name="ot")
                        nc.vector.tensor_scalar_mul(ot, pp[:, 0:NCLS - 1], rec)
                        nc.sync.dma_start(out=outd[qs, :], in_=ot)
    nc.finalize()
    return nc


_NC_CACHE = {}


def _get_nc(key):
    if key not in _NC_CACHE:
        NS, NQ, IN, EMB, NCLS = key
        _NC_CACHE[key] = build_nc(NS, NQ, IN, EMB, NCLS)
    return _NC_CACHE[key]


def _x_layout(x, kch):
    """[NV, IN] fp32 -> [NV/512, 128, KCH, 512] bf16 so each 512-row encoder
    block is one contiguous 1MiB DMA: H[nb,p,k,v] = x[nb*512+v, k*128+p]."""
    nv, in_dim = x.shape
    h = x.reshape(nv // 512, 512, kch, 128).transpose(0, 3, 2, 1)
    return np.ascontiguousarray(h.astype(ml_dtypes.bfloat16))


def _prep_inputs(support, query, W, b, support_labels, num_classes, n_cores,
                 shard_support=True):
    ncls = int(num_classes)
    bf = ml_dtypes.bfloat16
    support = np.asarray(support, np.float32)
    query = np.asarray(query, np.float32)
    W = np.asarray(W, np.float32)
    in_dim, emb = W.shape
    kch, mch = in_dim // 128, emb // 128
    ns = support.shape[0]
    # W[m, p, k, e] = W[k*128+p, m*128+e]
    Wh = np.ascontiguousarray(
        W.reshape(kch, 128, mch, 128).transpose(2, 1, 0, 3).astype(bf))
    # b[p, m] = b[m*128+p]
    bh = np.ascontiguousarray(np.asarray(b, np.float32).reshape(mch, 128).T)
    labels = np.asarray(support_labels).astype(np.int64)
    oh = np.zeros((ns, ncls + 1), dtype=bf)
    oh[np.arange(ns), labels] = 1
    oh[:, ncls] = 1  # ones column -> softmax denominator
    # oh[p, c, h] = onehot[c*128+p, h]
    ohh = np.ascontiguousarray(
        oh.reshape(ns // 128, 128, ncls + 1).transpose(1, 0, 2))
    nq_shard = query.shape[0] // n_cores
    ns_shard = ns // n_cores if shard_support else ns
    qh_all = _x_layout(query, kch)  # [NQ/512, 128, KCH, 512]
    nbq = nq_shard // 512
    in_maps = []
    for i in range(n_cores):
        sup_i = support[i * ns_shard:(i + 1) * ns_shard] if shard_support else support
        in_maps.append({
            "supX": _x_layout(sup_i, kch),
            "qX": np.ascontiguousarray(qh_all[i * nbq:(i + 1) * nbq]),
            "W": Wh,
            "b": bh,
            "onehot": ohh,
        })
    return in_maps


def _run(support, query, W, b, support_labels, num_classes, trace=False):
    ncls = int(num_classes)
    key = (support.shape[0], query.shape[0] // N_CORES, support.shape[1],
           W.shape[1], ncls + 1)
    nc = _get_nc(key)
    in_maps = _prep_inputs(support, query, W, b, support_labels, ncls, N_CORES)
    res = run_bass_kernel_spmd(nc, in_maps, list(range(N_CORES)), trace=trace)
    out = np.concatenate([r["out"] for r in res.results], axis=0)
    return out.astype(np.float32), res


def kernel(support, query, W, b, support_labels, num_classes):
    out, _ = _run(support, query, W, b, support_labels, num_classes, trace=False)
    return out
